# revision 15
# baseline (speedup 1.0000x reference)
"""Trainium2 Bass kernel for nn_MemoryModel (scatter_memory, 8 cores).

Math (per stage): the 8-point Gauss-Legendre quadrature over matrix
polynomials collapses algebraically:

  integral = V*S0 - REG*U*S1 + REG^2*P*S1 + REG^2/2*Q*S2
  with V = X - REG*(L@X), U = D@V, W1 = L@V, P = D@W1, Q = D@U
  and moments S_j = sum_k w_k t_k^j exp(dA t_k)   (elementwise [n,H])
  As_bar @ M' = M' - REG*(D@M') + REG^2*(D@(L@M')) + REG^2/2*(D@(D@M'))
  (M' = m_gather * At_bar)

Heavy passes per stage (k-outer, PSUM-batched over the 8 node tiles):
  L1: L@[X|M'] -> [LX|Y1]; L2: L@[V] -> W1; D1: D@[V|M'|Y1] -> [U|UM|T1];
  D2: D@[W1|U|UM] -> [P|Q|T2].

Sharding: H=128 column-sharded 8 ways (16 cols/core); the [1024,1024]
operators are replicated bf16. The memory-table gathers are done host-side
(numpy fancy-index) and shipped as a small packed param. One mid-kernel
AllGather carries stage-1 output c1^T to all cores for stage 2; a tiny
dummy AllGather issued as the first instruction absorbs the cross-core
rendezvous/barrier under stage-1 compute.

The quadrature moments are computed with ONE big exp (E_k = exp(t_k*dA),
[128, 8k, 128qh]) plus weighted k-accumulation chains, instead of 24
separate biased exps + GpSimd adds.
"""
import os
import sys

import numpy as np

for _p in ("/opt/trn_rl_repo", "/root/.axon_site/_ro/trn_rl_repo"):
    if os.path.isdir(_p) and _p not in sys.path:
        sys.path.insert(0, _p)

import ml_dtypes  # noqa: E402
import concourse.bass as bass  # noqa: E402
import concourse.bacc as bacc  # noqa: E402
import concourse.mybir as mybir  # noqa: E402
import concourse.tile as tile  # noqa: E402
from concourse.bass_utils import run_bass_kernel_spmd  # noqa: E402

F32 = mybir.dt.float32
F16 = mybir.dt.float16
BF16 = mybir.dt.bfloat16
AF = mybir.ActivationFunctionType
OP = mybir.AluOpType
BF = ml_dtypes.bfloat16

NA, H, DIN, E, NN, ED = 1024, 128, 172, 256, 100000, 1
KD = DIN + 2 * ED  # 174
REG = 0.1
REG2 = REG * REG
NCORES = 8
HS = 16  # H columns per core
NQ = 8  # node tiles (1024/128)

_gl_nodes = [-0.1834346424956498, -0.525532409916329, -0.7966664774136267,
             -0.9602898564975363, 0.1834346424956498, 0.525532409916329,
             0.7966664774136267, 0.9602898564975363]
_gl_w = [0.362683783378362, 0.3137066458778873, 0.2223810344533745,
         0.1012285362903763] * 2
T_NODES = [0.5 * (x + 1.0) for x in _gl_nodes]
T_W = [0.5 * w for w in _gl_w]
# moment-chain coefficients (REG^2/2 folded into S2)
C0 = [w for w in T_W]
C1 = [w * t for w, t in zip(T_W, T_NODES)]
C2 = [w * t * t * (REG2 / 2) for w, t in zip(T_W, T_NODES)]

_BUILD_CACHE = {}

# packA column map
PK_BT = 0      # b_tune
PK_LNH = 1     # 0.5*ln(H)
PK_BBC = 2     # bbc1 (17) | bbc2 (17)
PK_NEGA = 36   # negA1 (16) | negA2 (16)
PK_T = 68      # t_nodes (8)
PK_ID = 76     # identity (128)
PKA_N = 204


def _pin_act_table_set():
    """Restrict walrus's ACT-table choice to natural_log_exp_and_others +
    gelu so the kernel's exp/ln/gelu mix never ping-pongs table loads."""
    if os.environ.get("BASS_ACT_ROOT_JSON_PATH"):
        return
    try:
        import glob
        import json
        import tempfile

        import neuronxcc

        pwp = os.path.join(os.path.dirname(neuronxcc.__file__), "pwp",
                           "pwp_bin_trainium")
        info = json.load(open(os.path.join(pwp, "act_info.json")))
        keep_names = ["natural_log_exp_and_others",
                      "gelu_apprx_tanh_and_others"]
        keep = [s for s in info["act_func_sets"] if s["name"] in keep_names]
        keep.sort(key=lambda s: keep_names.index(s["name"]))
        if len(keep) != len(keep_names):
            return
        d = tempfile.mkdtemp(prefix="act_root_")
        for f in glob.glob(os.path.join(pwp, "*")):
            dst = os.path.join(d, os.path.basename(f))
            if not os.path.exists(dst):
                os.symlink(f, dst)
        out = dict(info)
        out["act_func_sets"] = keep
        patched = os.path.join(d, "act_info.json")
        os.unlink(patched)
        with open(patched, "w") as fh:
            json.dump(out, fh)
        import concourse.hw_specs as hw_specs

        tables = {
            s["name"]: {AF.from_pwp(v) for v in s["act"].keys()} for s in keep
        }

        def _tables(arch, _t=tables):
            return _t

        hw_specs.get_activation_tables = _tables
        bacc.get_activation_tables = _tables
        os.environ["BASS_ACT_ROOT_JSON_PATH"] = patched
    except Exception:
        pass


def build_bass():
    if "nc" in _BUILD_CACHE:
        return _BUILD_CACHE["nc"]
    _pin_act_table_set()
    nc = bacc.Bacc("TRN2", target_bir_lowering=False, debug=False,
                   num_devices=NCORES)
    dp = nc.declare_dram_parameter

    packA = dp("packA", [128, PKA_N], F32, isOutput=False)
    packB = dp("packB", [128, 35], BF16, isOutput=False)
    lt_hi = dp("lt_hi", [128, NQ * 1024], BF16, isOutput=False)
    dt_hi = dp("dt_hi", [128, NQ * 1024], BF16, isOutput=False)
    xsT_a = dp("xsT_a", [128, 1024], BF16, isOutput=False)
    xsT_b = dp("xsT_b", [KD - 128, 1024], BF16, isOutput=False)
    wtune_a = dp("wtune_a", [128, 128], BF16, isOutput=False)
    wtune_b = dp("wtune_b", [KD - 128, 128], BF16, isOutput=False)
    mh = dp("mh", [128, NQ, 2 * HS], F32, isOutput=False)  # host-gathered

    c1o = dp("c1o", [128, NQ, HS], F32, isOutput=True)
    c2o = dp("c2o", [128, NQ, HS], F32, isOutput=True)

    DBG = bool(os.environ.get("KDBG"))
    if DBG:
        dbg = {
            "d_ztT": dp("d_ztT", [128, 1024], F32, isOutput=True),
            "d_BD": dp("d_BD", [128, NQ, 17], F32, isOutput=True),
            "d_deltap": dp("d_deltap", [128, NQ, 1], F32, isOutput=True),
            "d_dA": dp("d_dA", [128, NQ, HS], F32, isOutput=True),
            "d_S0": dp("d_S0", [128, 128], F32, isOutput=True),
            "d_S1n": dp("d_S1n", [128, 128], F32, isOutput=True),
            "d_R0": dp("d_R0", [128, NQ, 2 * HS], F32, isOutput=True),
            "d_R1": dp("d_R1", [128, NQ, 3 * HS], F32, isOutput=True),
            "d_R2": dp("d_R2", [128, NQ, 3 * HS], F32, isOutput=True),
            "d_OUT2": dp("d_OUT2", [128, NQ, 3 * HS], F32, isOutput=True),
            "d_Mf": dp("d_Mf", [128, NQ, HS], F32, isOutput=True),
            "d_gacc": dp("d_gacc", [128, NQ, HS], F32, isOutput=True),
        }

    # collective bounce buffers
    ag_in = nc.dram_tensor("ag_in", [HS, 1024], F16)
    ag_out = nc.dram_tensor("ag_out", [128, 1024], F16, addr_space="Shared")
    dum_in = nc.dram_tensor("dum_in", [1, 128], F32)
    dum_out = nc.dram_tensor("dum_out", [NCORES, 128], F32,
                             addr_space="Shared")

    with tile.TileContext(nc) as tc:
        with tc.tile_pool(name="const", bufs=1) as cst, \
             tc.tile_pool(name="work", bufs=1) as wk, \
             tc.tile_pool(name="ph", bufs=4, space="PSUM") as ph, \
             tc.tile_pool(name="pz", bufs=2, space="PSUM") as pz, \
             tc.tile_pool(name="psml", bufs=1, space="PSUM") as psml, \
             tc.tile_pool(name="pt", bufs=1, space="PSUM") as pt:

            # --- pre-warm the collective path: tiny dummy AllGather with
            # zero data dependencies (reads an uninitialized scratch dram
            # tensor -- values are irrelevant) issued as the very first
            # instruction, so the cross-core rendezvous barrier runs under
            # stage-1 compute.
            nc.gpsimd.collective_compute(
                "AllGather", OP.bypass,
                replica_groups=[list(range(NCORES))],
                ins=[dum_in[:]], outs=[dum_out[:]],
            )

            # ---------- constant loads (order = DMA priority) ----------
            pack_sb = cst.tile([128, PKA_N], F32, tag="packA")
            xsT_a_sb = cst.tile([128, 1024], BF16, tag="xsTa")
            xsT_b_sb = cst.tile([KD - 128, 1024], BF16, tag="xsTb")
            wtune_a_sb = cst.tile([128, 128], BF16, tag="wta")
            wtune_b_sb = cst.tile([KD - 128, 128], BF16, tag="wtb")
            packb_sb = cst.tile([128, 35], BF16, tag="packB")
            mh_sb = cst.tile([128, NQ, 2 * HS], F32, tag="mh")
            lt_sb = cst.tile([128, NQ, 1024], BF16, tag="lt_hi")
            dt_sb = cst.tile([128, NQ, 1024], BF16, tag="dt_hi")

            nc.sync.dma_start(out=pack_sb[:], in_=packA[:])
            nc.sync.dma_start(out=xsT_a_sb[:], in_=xsT_a[:])
            nc.sync.dma_start(out=wtune_a_sb[:], in_=wtune_a[:])
            nc.sync.dma_start(out=xsT_b_sb[:], in_=xsT_b[:])
            nc.sync.dma_start(out=wtune_b_sb[:], in_=wtune_b[:])
            nc.sync.dma_start(out=packb_sb[:], in_=packB[:])
            nc.sync.dma_start(out=mh_sb[:], in_=mh[:])
            for k in range(NQ):
                nc.sync.dma_start(out=lt_sb[:, k, :],
                                  in_=lt_hi[:, k * 1024:(k + 1) * 1024])
            for k in range(NQ):
                nc.sync.dma_start(out=dt_sb[:, k, :],
                                  in_=dt_hi[:, k * 1024:(k + 1) * 1024])

            wb_sb = [packb_sb[:, 17 * s:17 * (s + 1)] for s in range(2)]
            ones_ap = packb_sb[:, 34:35]
            ident_ap = pack_sb[:, PK_ID:PK_ID + 128]
            bbc = [pack_sb[:, PK_BBC + 17 * s:PK_BBC + 17 * (s + 1)]
                   for s in range(2)]
            negA = [pack_sb[:, PK_NEGA + HS * s:PK_NEGA + HS * (s + 1)]
                    for s in range(2)]
            tvec = pack_sb[:, PK_T:PK_T + 8]

            # zt^T = W_tune^T @ x_in^T + b_tune  [128 H, 1024 nodes] f32
            ztT = wk.tile([128, 1024], F32, tag="ztT")
            for hhalf in range(2):
                ps = pz.tile([128, 512], F32, tag="zt")
                cols = slice(hhalf * 512, (hhalf + 1) * 512)
                nc.tensor.matmul(ps[:], lhsT=wtune_a_sb[:],
                                 rhs=xsT_a_sb[:, cols], start=True, stop=False)
                nc.tensor.matmul(ps[:], lhsT=wtune_b_sb[:],
                                 rhs=xsT_b_sb[:, cols], start=False, stop=True)
                nc.vector.tensor_scalar(out=ztT[:, cols], in0=ps[:],
                                        scalar1=pack_sb[:, PK_BT:PK_BT + 1],
                                        scalar2=None, op0=OP.add)

            c1T_full = wk.tile([128, 1024], F16, tag="c1T_full")
            u2T = wk.tile([128, 1024], F32, tag="u2T")
            gtmp = wk.tile([128, 1024], F32, tag="gtmp")

            couts = (c1o, c2o)

            for s in range(2):  # the two SSM stages
                if s == 0:
                    base = ztT
                else:
                    # u2 = zt + gelu(c1), halved so the add overlaps gelu
                    for h2 in range(2):
                        nco = slice(h2 * 512, (h2 + 1) * 512)
                        nc.scalar.activation(gtmp[:, nco], c1T_full[:, nco],
                                             AF.Gelu_apprx_tanh)
                        nc.vector.tensor_tensor(out=u2T[:, nco],
                                                in0=ztT[:, nco],
                                                in1=gtmp[:, nco], op=OP.add)
                    base = u2T

                # ---- small pipeline (transposed land, whole-tile) ----
                baseS = wk.tile([128, 1024], BF16, tag=f"baseS{s}")
                sq = wk.tile([128, 1024], BF16, tag=f"sq{s}")
                nc.vector.tensor_copy(out=baseS[:], in_=base[:])
                nc.scalar.activation(sq[:], base[:], AF.Square)

                # B/delta matmuls + ss (sum zt^2) per node tile
                psb = psml.tile([128, NQ, 18], F32, tag="sp")
                for q in range(NQ):
                    qs = slice(q * 128, (q + 1) * 128)
                    nc.tensor.matmul(psb[:, q, 0:17], lhsT=baseS[:, qs],
                                     rhs=wb_sb[s], start=True, stop=True)
                    nc.tensor.matmul(psb[:, q, 17:18], lhsT=sq[:, qs],
                                     rhs=ones_ap, start=True, stop=True)

                # rinv = sqrt(H)/sqrt(ss) via exp/ln
                lnss = wk.tile([128, NQ, 1], F32, tag=f"lnss{s}")
                rinv = wk.tile([128, NQ, 1], F32, tag=f"rinv{s}")
                nc.scalar.activation(lnss[:], psb[:, :, 17:18], AF.Ln)
                nc.scalar.activation(rinv[:], lnss[:], AF.Exp, scale=-0.5,
                                     bias=pack_sb[:, PK_LNH:PK_LNH + 1])

                # BD = psb*rinv + [b_B|b_dt]
                BD = wk.tile([128, NQ, 17], F32, tag=f"BD{s}")
                nc.vector.tensor_tensor(
                    out=BD[:], in0=psb[:, :, 0:17],
                    in1=rinv[:].to_broadcast([128, NQ, 17]), op=OP.mult)
                nc.vector.tensor_tensor(
                    out=BD[:], in0=BD[:],
                    in1=bbc[s].unsqueeze(1).to_broadcast([128, NQ, 17]),
                    op=OP.add)

                # delta = softplus(BD[...,16]) = ln(1+exp)
                esp = wk.tile([128, NQ, 1], F32, tag=f"esp{s}")
                deltap = wk.tile([128, NQ, 1], F32, tag=f"deltap{s}")
                nc.scalar.activation(esp[:], BD[:, :, 16:17], AF.Exp)
                nc.scalar.activation(deltap[:], esp[:], AF.Ln, bias=1.0)

                # R0 = [X | M'] bf16; X = B*delta; M' = m_gather*At
                R0 = wk.tile([128, NQ, 2 * HS], BF16, tag=f"R0{s}")
                dA = wk.tile([128, NQ, HS], F32, tag=f"dA{s}")
                At = wk.tile([128, NQ, HS], F32, tag=f"At{s}")
                Mf = wk.tile([128, NQ, HS], F32, tag=f"Mf{s}")
                nc.vector.tensor_tensor(
                    out=R0[:, :, 0:HS], in0=BD[:, :, 0:16],
                    in1=deltap[:].to_broadcast([128, NQ, HS]), op=OP.mult)
                nc.gpsimd.tensor_tensor(
                    out=dA[:],
                    in0=deltap[:].to_broadcast([128, NQ, HS]),
                    in1=negA[s].unsqueeze(1).to_broadcast([128, NQ, HS]),
                    op=OP.mult)
                nc.scalar.activation(At[:], dA[:], AF.Exp)
                nc.gpsimd.tensor_tensor(
                    out=Mf[:], in0=mh_sb[:, :, s * HS:(s + 1) * HS],
                    in1=At[:], op=OP.mult)
                nc.vector.tensor_copy(out=R0[:, :, HS:2 * HS], in_=Mf[:])

                # ---- moments: E = exp(t_k * dA)  [128, 8k, 128qh] ----
                dAt = wk.tile([128, 8, 128], BF16, tag=f"dAt{s}")
                Em = wk.tile([128, 8, 128], BF16, tag=f"Em{s}")
                dA_v = dA[:].rearrange("p q h -> p (q h)").unsqueeze(1)
                nc.vector.tensor_tensor(
                    out=dAt[:], in0=dA_v.to_broadcast([128, 8, 128]),
                    in1=tvec.unsqueeze(2).to_broadcast([128, 8, 128]),
                    op=OP.mult)
                nc.scalar.activation(Em[:], dAt[:], AF.Exp)

                # S_j chains (vector STT; gpsimd lacks immediate-scalar ops)
                # S1 is pre-scaled twice for the combine: S1n=-REG*S1,
                # S1p=REG^2*S1 (folded into the chain coefficients).
                Sm = [wk.tile([128, 128], F32, tag=f"S{j}{s}",
                              name=f"S{j}{s}") for j in range(4)]
                C1n = [-REG * c for c in C1]
                for j, cj in enumerate((C0, C1n, C2)):
                    nc.vector.tensor_scalar(out=Sm[j][:], in0=Em[:, 0, :],
                                            scalar1=float(cj[0]),
                                            scalar2=None, op0=OP.mult)
                    for k in range(1, 8):
                        nc.vector.scalar_tensor_tensor(
                            out=Sm[j][:], in0=Em[:, k, :],
                            scalar=float(cj[k]), in1=Sm[j][:],
                            op0=OP.mult, op1=OP.add)
                # S1p = REG^2 * S1 = -REG * S1n
                nc.vector.tensor_scalar(out=Sm[3][:], in0=Sm[1][:],
                                        scalar1=-REG, scalar2=None,
                                        op0=OP.mult)
                S0 = Sm[0][:].rearrange("p (q h) -> p q h", q=NQ)
                S1n = Sm[1][:].rearrange("p (q h) -> p q h", q=NQ)
                S2 = Sm[2][:].rearrange("p (q h) -> p q h", q=NQ)
                S1p = Sm[3][:].rearrange("p (q h) -> p q h", q=NQ)

                # ---- heavy passes (k-outer, PSUM-batched over q) ----
                def hpass(op_sb, rhs_ap, ncols, tag):
                    # k-outer streaming with all 8 node-tile accumulation
                    # regions batched in ONE psum bank. start=True clears
                    # has_written for the whole bank, so only the very
                    # first matmul of the pass sets it; every other region
                    # first-touch overwrites (bit clear) then accumulates.
                    ps = ph.tile([128, NQ, 3 * HS], F32, tag="hv",
                                 name=tag)[:, :, 0:ncols]
                    for kt in range(NQ):
                        for q in range(NQ):
                            nc.tensor.matmul(
                                ps[:, q, :],
                                lhsT=op_sb[:, kt, q * 128:(q + 1) * 128],
                                rhs=rhs_ap(kt),
                                start=(kt == 0 and q == 0),
                                stop=(kt == NQ - 1 and q == NQ - 1),
                            )
                    return ps

                # L1: L @ [X|M'] -> [LX | Y1]
                ps1 = hpass(lt_sb, lambda kt: R0[:, kt, :], 2 * HS, f"L1_{s}")
                R1 = wk.tile([128, NQ, 3 * HS], BF16, tag=f"R1{s}")  # V|M|Y1
                nc.vector.scalar_tensor_tensor(
                    out=R1[:, :, 0:HS], in0=ps1[:, :, 0:HS], scalar=-REG,
                    in1=R0[:, :, 0:HS], op0=OP.mult, op1=OP.add)
                nc.vector.tensor_copy(out=R1[:, :, HS:2 * HS],
                                      in_=R0[:, :, HS:2 * HS])
                nc.vector.tensor_copy(out=R1[:, :, 2 * HS:3 * HS],
                                      in_=ps1[:, :, HS:2 * HS])

                # L2: L @ V -> W1
                R2 = wk.tile([128, NQ, 3 * HS], BF16, tag=f"R2{s}")  # W1|U|UM
                ps2 = hpass(lt_sb, lambda kt: R1[:, kt, 0:HS], HS, f"L2_{s}")
                nc.vector.tensor_copy(out=R2[:, :, 0:HS], in_=ps2[:])

                # D1: D @ [V|M'|Y1] -> [U | UM | T1]
                ps3 = hpass(dt_sb, lambda kt: R1[:, kt, :], 3 * HS, f"D1_{s}")
                nc.vector.tensor_copy(out=R2[:, :, HS:3 * HS],
                                      in_=ps3[:, :, 0:2 * HS])

                # D2: D @ [W1|U|UM] -> [P | Q | T2]
                ps4 = hpass(dt_sb, lambda kt: R2[:, kt, :], 3 * HS, f"D2_{s}")
                OUT2 = wk.tile([128, NQ, 3 * HS], BF16, tag=f"OUT2{s}")
                nc.vector.tensor_copy(out=OUT2[:], in_=ps4[:])

                # ---- combine ----
                acc = wk.tile([128, NQ, HS], F32, tag=f"acc{s}")
                gacc = wk.tile([128, NQ, HS], F32, tag=f"gacc{s}")
                # a-chain (vector): M' - REG*UM + REG^2*T1 + REG^2/2*T2
                nc.vector.scalar_tensor_tensor(
                    out=acc[:], in0=ps3[:, :, HS:2 * HS], scalar=-REG,
                    in1=Mf[:], op0=OP.mult, op1=OP.add)
                nc.vector.scalar_tensor_tensor(
                    out=acc[:], in0=ps3[:, :, 2 * HS:3 * HS], scalar=REG2,
                    in1=acc[:], op0=OP.mult, op1=OP.add)
                nc.vector.scalar_tensor_tensor(
                    out=acc[:], in0=OUT2[:, :, 2 * HS:3 * HS], scalar=REG2 / 2,
                    in1=acc[:], op0=OP.mult, op1=OP.add)
                # b-chain (gpsimd, SBUF + plain TT only):
                #   V*S0 + U*S1n + P*S1p + Q*S2'
                pb1 = wk.tile([128, NQ, HS], F32, tag=f"pb1{s}")
                pb2 = wk.tile([128, NQ, HS], F32, tag=f"pb2{s}")
                nc.gpsimd.tensor_tensor(out=gacc[:], in0=R1[:, :, 0:HS],
                                        in1=S0, op=OP.mult)
                nc.gpsimd.tensor_tensor(out=pb1[:], in0=R2[:, :, HS:2 * HS],
                                        in1=S1n, op=OP.mult)
                nc.gpsimd.tensor_tensor(out=pb2[:], in0=OUT2[:, :, 0:HS],
                                        in1=S1p, op=OP.mult)
                nc.gpsimd.tensor_tensor(out=gacc[:], in0=gacc[:], in1=pb1[:],
                                        op=OP.add)
                nc.gpsimd.tensor_tensor(out=pb1[:], in0=OUT2[:, :, HS:2 * HS],
                                        in1=S2, op=OP.mult)
                nc.gpsimd.tensor_tensor(out=gacc[:], in0=gacc[:], in1=pb2[:],
                                        op=OP.add)
                nc.gpsimd.tensor_tensor(out=gacc[:], in0=gacc[:], in1=pb1[:],
                                        op=OP.add)
                if DBG and s == 0:
                    nc.sync.dma_start(out=dbg["d_ztT"][:], in_=ztT[:])
                    nc.sync.dma_start(out=dbg["d_BD"][:], in_=BD[:])
                    nc.sync.dma_start(out=dbg["d_deltap"][:], in_=deltap[:])
                    nc.sync.dma_start(out=dbg["d_dA"][:], in_=dA[:])
                    nc.sync.dma_start(out=dbg["d_S0"][:], in_=Sm[0][:])
                    nc.sync.dma_start(out=dbg["d_S1n"][:], in_=Sm[1][:])
                    nc.sync.dma_start(out=dbg["d_Mf"][:], in_=Mf[:])
                    nc.sync.dma_start(out=dbg["d_gacc"][:], in_=gacc[:])
                    for nm, t in (("d_R0", R0), ("d_R1", R1), ("d_R2", R2),
                                  ("d_OUT2", OUT2)):
                        tmpd = wk.tile(list(t[:].shape), F32, tag=nm, name=nm)
                        nc.vector.tensor_copy(out=tmpd[:], in_=t[:])
                        nc.sync.dma_start(out=dbg[nm][:], in_=tmpd[:])

                # final fold + output
                nc.vector.tensor_tensor(out=acc[:], in0=acc[:], in1=gacc[:],
                                        op=OP.add)
                nc.sync.dma_start(out=couts[s][:], in_=acc[:])

                if s == 0:
                    # transpose c1 -> [16, 1024] f16, AllGather, download
                    c1Ts = wk.tile([HS, 1024], F16, tag="c1Ts")
                    for h4 in range(2):
                        ptr = pt.tile([HS, 4, 128], F32, tag="trp",
                                      name=f"trp{h4}")
                        for qi in range(4):
                            q = h4 * 4 + qi
                            nc.tensor.transpose(ptr[:, qi, :], acc[:, q, :],
                                                ident_ap)
                        nc.scalar.activation(
                            c1Ts[:, h4 * 512:(h4 + 1) * 512],
                            ptr[:].rearrange("p q n -> p (q n)"), AF.Copy)
                    nc.sync.dma_start(out=ag_in[:], in_=c1Ts[:])
                    nc.gpsimd.collective_compute(
                        "AllGather", OP.bypass,
                        replica_groups=[list(range(NCORES))],
                        ins=[ag_in[:]], outs=[ag_out[:]],
                    )
                    nc.sync.dma_start(out=c1T_full[:, 0:512],
                                      in_=ag_out[:, 0:512])
                    nc.sync.dma_start(out=c1T_full[:, 512:1024],
                                      in_=ag_out[:, 512:1024])

    nc.compile()
    _BUILD_CACHE["nc"] = nc
    return nc


def _pack_kt(a_T):
    """[1024, 1024] (k-major rows) -> [128, 8*1024] partition-packed bf16."""
    return a_T.reshape(NQ, 128, 1024).transpose(1, 0, 2).reshape(128, NQ * 1024)


def kernel(**inputs):
    out, _ = _run(inputs, trace=False)
    return out


def _run(inputs, trace=False, trace_kwargs=None):
    inp = {k: np.asarray(v) for k, v in inputs.items()}
    L = inp["L_agg"].astype(np.float32)
    D = inp["delta_L_agg"].astype(np.float32)
    x_sub = inp["x_sub"].astype(np.float32)
    m1 = inp["m1_vec"].astype(np.float32)
    m2 = inp["m2_vec"].astype(np.float32)
    names = inp["names_table"].astype(np.float32)
    rms1 = inp["rms1_scale"].astype(np.float32)
    rms2 = inp["rms2_scale"].astype(np.float32)
    W_tune = inp["W_tune"].astype(np.float32)
    b_tune = inp["b_tune"].astype(np.float32)
    W_B1 = inp["W_B1"].astype(np.float32)
    b_B1 = inp["b_B1"].astype(np.float32)
    W_B2 = inp["W_B2"].astype(np.float32)
    b_B2 = inp["b_B2"].astype(np.float32)
    W_dt = inp["W_dt"].astype(np.float32)
    b_dt = inp["b_dt"].astype(np.float32)
    A1 = inp["A_log_1"].astype(np.float32)
    A2 = inp["A_log_2"].astype(np.float32)
    tsrc = np.asarray(inp["target_src"]).astype(np.int64)
    tdst = np.asarray(inp["target_dst"]).astype(np.int64)
    aids = np.asarray(inp["active_input_ids"]).astype(np.int64)

    # x_in = [x_sub | neigh]
    neigh = np.zeros((NA, 2 * ED), np.float32)
    neigh[:E, :ED] = names[tsrc]
    neigh[:E, ED:] = names[tdst]
    neigh[E:2 * E, :ED] = names[tdst]
    neigh[E:2 * E, ED:] = names[tsrc]
    x_in = np.concatenate([x_sub, neigh], axis=1)  # [1024, 174]
    xsT = np.ascontiguousarray(x_in.T)  # [174, 1024]

    lt_hi = _pack_kt(np.ascontiguousarray(L.T).astype(BF))
    dt_hi = _pack_kt(np.ascontiguousarray(D.T).astype(BF))

    negA1_full = -np.exp(A1)  # [128]
    negA2_full = -np.exp(A2)

    # host-side memory gathers: [1024, H] -> node-packed [128, 8, H]
    m1g = m1[aids].reshape(NQ, 128, H).transpose(1, 0, 2)  # [128,8,H]
    m2g = m2[aids].reshape(NQ, 128, H).transpose(1, 0, 2)

    common = {
        "lt_hi": lt_hi, "dt_hi": dt_hi,
        "xsT_a": xsT[:128].astype(BF),
        "xsT_b": np.ascontiguousarray(xsT[128:]).astype(BF),
        "wtune_a": W_tune[:128].astype(BF),
        "wtune_b": np.ascontiguousarray(W_tune[128:]).astype(BF),
    }

    in_maps = []
    for c in range(NCORES):
        hs = slice(c * HS, (c + 1) * HS)
        # rms scales folded into the B/dt weights (scale enters linearly
        # after the 1/rms normalization)
        wb1c = (np.concatenate([W_B1[:, hs], W_dt], axis=1)
                * rms1[:, None]).astype(BF)
        wb2c = (np.concatenate([W_B2[:, hs], W_dt], axis=1)
                * rms2[:, None]).astype(BF)
        packa = np.zeros((128, PKA_N), np.float32)
        packa[:, PK_BT] = b_tune
        packa[:, PK_LNH] = 0.5 * np.log(H)
        packa[:, PK_BBC:PK_BBC + 17] = np.concatenate([b_B1[hs], b_dt])
        packa[:, PK_BBC + 17:PK_BBC + 34] = np.concatenate([b_B2[hs], b_dt])
        packa[:, PK_NEGA:PK_NEGA + HS] = negA1_full[hs]
        packa[:, PK_NEGA + HS:PK_NEGA + 2 * HS] = negA2_full[hs]
        packa[:, PK_T:PK_T + 8] = np.asarray(T_NODES, np.float32)
        packa[:, PK_ID:PK_ID + 128] = np.eye(128, dtype=np.float32)
        packb = np.concatenate([wb1c, wb2c, np.ones((128, 1), BF)], axis=1)
        mhc = np.concatenate([m1g[:, :, hs], m2g[:, :, hs]], axis=2)
        in_maps.append({
            **common,
            "packA": packa, "packB": np.ascontiguousarray(packb),
            "mh": np.ascontiguousarray(mhc),
        })

    nc = build_bass()
    res = run_bass_kernel_spmd(nc, in_maps, core_ids=list(range(NCORES)),
                               trace=trace, **(trace_kwargs or {}))

    out = np.zeros((2, NA, H), np.float32)
    for c in range(NCORES):
        hs = slice(c * HS, (c + 1) * HS)
        out[0][:, hs] = res.results[c]["c1o"].transpose(1, 0, 2).reshape(NA, HS)
        out[1][:, hs] = res.results[c]["c2o"].transpose(1, 0, 2).reshape(NA, HS)
    return out, res
